# revision 1
# baseline (speedup 1.0000x reference)
"""BatchAll triplet loss (multi-module variant) on 8 Trainium2 NeuronCores.

Math: labels = [0..191, 0..191] -- every label appears exactly twice, so each
anchor i has exactly ONE valid positive j = (i+192) % 384.  The (i,j,k) cubic
triplet tensor therefore collapses to an (i,k) problem:

    loss_terms[i,k] = relu(d(i, p(i)) - d(i,k) + margin) * pm[i,k] * valid[i,k]
    out = sum(loss_terms) / (count(loss_terms > EPS) + EPS)

where valid excludes k in {i, p(i)} and pm = tile(weight, (2,2)).

With unit-normalized embeddings, d(i,k) = sqrt(relu(2 - 2*G[i,k]*rn_i*rn_k))
where G is the raw Gram matrix and rn = 1/||e||.  (The reference's distance
uses the normalized Gram's diagonal, which equals 1 up to 1e-7 rounding; the
constant 2 is within mutual fp32 noise.)

Weighting trick: with pmn = -pm, relu(dpos+m-d)*pm == max((d-(dpos+m))*pmn, 0)
and count(lw > EPS) == count((d-(dpos+m))*pmn > EPS) since EPS > 0.

Sharding: anchors i are blocked over the 8 cores (48 each).  Each core receives
the full embedding set TRANSPOSED and ROTATED so that its anchor slab lands at
local columns 0..47 and the positives at columns 192..239 -- one shared NEFF,
per-core data.  Each core emits its partial [sum, count]; the host reduces.

Hardware notes driving the structure (from NTFF traces):
- instructions carry at most ONE semaphore wait, so each op depends on at most
  one producer engine (Bacc legalizes violations with costly event-sem nops).
- engines execute in order: an op waiting on late data stalls everything
  behind it on that engine, so per-engine program order = readiness order.
- the PE is HAM-throttled cold (~2-4x); a few dummy matmuls during the DMA
  phase warm it before the real matmuls run.
- only sync/scalar (HWDGE) and gpsimd (SWDGE, ~6us completion latency) can
  initiate DMAs; big loads are split across the two HWDGE rings.
- a (1,384) one-lane DVE reciprocal costs 2.5us; computing 1/norm on a
  (128,3) layout and PE-transposing to rows costs ~0.5us total.
- the ACT Sqrt table load (1.3us) is pulled off the critical path by a dummy
  sqrt issued while DMAs are in flight.
"""

import os
import sys

for _p in ("/opt/trn_rl_repo", "/root/.axon_site/_ro/trn_rl_repo"):
    if _p not in sys.path:
        sys.path.append(_p)

# The SPMD dispatch path (bass2jax.run_bass_via_pjrt) takes jax.devices(), so
# the axon platform must stay visible.  If jax has not been initialized yet and
# JAX_PLATFORMS would hide it (e.g. "cpu"), clear the restriction.
if "jax" not in sys.modules and os.environ.get("JAX_PLATFORMS") in ("cpu",):
    del os.environ["JAX_PLATFORMS"]

import numpy as np

import concourse.bass as bass
import concourse.tile as tile
from concourse import mybir
from concourse.bacc import Bacc
from concourse.bass_utils import run_bass_kernel_spmd

F32 = mybir.dt.float32
ALU = mybir.AluOpType
ACT = mybir.ActivationFunctionType

B = 192          # batch (distinct labels)
N = 2 * B        # embeddings
D = 512          # embedding dim
NCORES = 8
S = N // NCORES  # anchors per core (48)
MARGIN = 0.1
EPS = 1e-8
N_WARMUP = 4     # dummy matmuls to bring the PE out of HAM throttle


def build_nc() -> bass.Bass:
    nc = Bacc()

    embt = nc.dram_tensor("embt", [D, N], F32, kind="ExternalInput")
    embr = nc.dram_tensor("embr", [N, D], F32, kind="ExternalInput")
    pmwn = nc.dram_tensor("pmwn", [S, N], F32, kind="ExternalInput")
    out = nc.dram_tensor("out", [1, 2], F32, kind="ExternalOutput")

    KC = D // 128   # contraction chunks for the Gram (4)
    RT = N // 128   # row-layout tiles / rn row chunks (3)

    with tile.TileContext(nc) as tc:
        with (
            tc.tile_pool(name="sb", bufs=1) as sb,
            tc.tile_pool(name="ps", bufs=1, space="PSUM") as ps,
        ):
            E, R = [], []
            for c in range(KC):
                e = sb.tile([128, N], F32, tag=f"E{c}")
                E.append(e)
            for t in range(RT):
                r = sb.tile([128, D], F32, tag=f"R{t}")
                R.append(r)
            pm = sb.tile([S, N], F32, tag="pm")

            # ---- loads: E chunks first so the Gram follows the PE warm-up
            #      with no idle gap (HAM re-throttles after ~1us idle) ----
            nc.sync.dma_start(out=E[0], in_=embt[0:128, :])
            nc.scalar.dma_start(out=E[2], in_=embt[256:384, :])
            nc.sync.dma_start(out=R[0], in_=embr[0:128, :])
            nc.scalar.dma_start(out=R[1], in_=embr[128:256, :])
            nc.sync.dma_start(out=E[1], in_=embt[128:256, :])
            nc.scalar.dma_start(out=E[3], in_=embt[384:512, :])
            nc.sync.dma_start(out=R[2], in_=embr[256:384, :])
            nc.gpsimd.dma_start(out=pm, in_=pmwn[:, :])   # needed late; SWDGE ok

            # ---- identity via iota on the (otherwise idle) gpsimd ----
            icol = sb.tile([128, 128], F32, tag="icol")
            nc.gpsimd.iota(icol, [[1, 128]], channel_multiplier=0,
                           allow_small_or_imprecise_dtypes=True)
            iprt = sb.tile([128, 1], F32, tag="iprt")
            nc.gpsimd.iota(iprt, [[0, 1]], channel_multiplier=1,
                           allow_small_or_imprecise_dtypes=True)
            ident = sb.tile([128, 128], F32, tag="ident")
            nc.gpsimd.tensor_scalar(ident, icol, iprt, None, op0=ALU.is_equal)

            # ---- warm-up scaffolding ----
            wtile = sb.tile([128, 256], F32, tag="wtile")
            nc.vector.memset(wtile, 1.0)
            ones_col = sb.tile([S, 1], F32, tag="ones_col")
            nc.vector.memset(ones_col, 1.0)
            ones_row = sb.tile([1, S], F32, tag="ones_row")
            nc.vector.memset(ones_row, 1.0)
            twos_col = sb.tile([S, 1], F32, tag="twos_col")
            nc.vector.memset(twos_col, 2.0)
            tdum = sb.tile([1, 1], F32, tag="tdum")
            nc.scalar.sqrt(tdum, wtile[0:1, 0:1])   # pull ACT sqrt table early

            wps = ps.tile([128, 256], F32, tag="wps")
            for _ in range(N_WARMUP):
                nc.tensor.matmul(wps, wtile[:, 0:128], wtile,
                                 start=True, stop=True)

            # ---- norms on DVE: ns_col[p,t] = ||emb[128t+p]||^2 ----
            ns_col = sb.tile([128, RT], F32, tag="ns_col")
            junk = sb.tile([128, D], F32, tag="junk")
            for t in range(RT):
                nc.vector.scalar_tensor_tensor(
                    junk, R[t], 1.0, R[t], op0=ALU.mult, op1=ALU.mult,
                    accum_out=ns_col[:, t:t + 1])
            nrm_col = sb.tile([128, RT], F32, tag="nrm_col")
            nc.scalar.sqrt(nrm_col, ns_col)
            rn_col = sb.tile([128, RT], F32, tag="rn_col")   # 1/||e||
            nc.vector.reciprocal(rn_col, nrm_col)

            # ---- Gram slab: G[a,k] = sum_d embt[d,a] * embt[d,k] ----
            g_ps = ps.tile([S, N], F32, tag="G")
            for c in range(KC):
                nc.tensor.matmul(g_ps, E[c][:, 0:S], E[c],
                                 start=(c == 0), stop=(c == KC - 1))

            # ---- -2*rn_a row scale of G (DVE work while PE transposes) ----
            rnam2 = sb.tile([S, 1], F32, tag="rnam2")        # -2 * rn[slab]
            nc.vector.tensor_scalar_mul(rnam2, rn_col[0:S, 0:1], -2.0)
            x1 = sb.tile([S, N], F32, tag="X1")              # -2 rn_a G
            nc.vector.tensor_scalar_mul(x1, g_ps, rnam2)

            # ---- rn to rows via PE transposes; copies split DVE/ACT ----
            rn_t = []
            for j in range(RT):
                rj_ps = ps.tile([1, 128], F32, tag=f"rnT{j}")
                nc.tensor.transpose(rj_ps, rn_col[:, j:j + 1], ident)
                rj = sb.tile([1, 128], F32, tag=f"rn_t{j}")
                if j == 1:
                    nc.scalar.copy(rj, rj_ps)      # gpsimd can't read PSUM
                else:
                    nc.vector.tensor_copy(rj, rj_ps)
                rn_t.append(rj)

            # ---- RB[a,k] = rn_k broadcast (rank-1, 128 cols per chunk) ----
            rb_ps = ps.tile([S, N], F32, tag="RB")
            for j in range(RT):
                nc.tensor.matmul(rb_ps[:, j * 128:(j + 1) * 128], ones_row,
                                 rn_t[j], start=True, stop=True)

            # ---- d2 = relu(2 - 2 * G * rn_a * rn_k) ----
            t1 = sb.tile([S, N], F32, tag="T1")              # -2 rn_a rn_k G
            nc.vector.tensor_mul(t1, x1, rb_ps)
            d2 = sb.tile([S, N], F32, tag="D2")
            nc.vector.tensor_scalar(d2, t1, 2.0, 0.0, op0=ALU.add, op1=ALU.max)
            dms = sb.tile([S, N], F32, tag="dms")
            nc.scalar.sqrt(dms, d2)

            # ---- positive distance straight from t1's diagonal block
            #      (pre-relu; d2_pos ~ 2 > 0 always): dpos = sqrt(t1_pos + 2),
            #      with the +2 folded into the sqrt bias ----
            dpb = sb.tile([S, S], F32, tag="dpb")
            t1pos = sb.tile([S, 1], F32, tag="t1pos")
            nc.vector.scalar_tensor_tensor(
                dpb, t1[:, B:B + S], 1.0, ident[0:S, 0:S], op0=ALU.mult,
                op1=ALU.mult, accum_out=t1pos)
            dpos = sb.tile([S, 1], F32, tag="dpos")
            nc.scalar.activation(dpos, t1pos, ACT.Sqrt, bias=twos_col, scale=1.0)
            dpos_m = sb.tile([S, 1], F32, tag="dpos_m")
            nc.vector.tensor_scalar_add(dpos_m, dpos, MARGIN)

            # ---- weighted triplet terms via the negated-weight trick ----
            lwpre = sb.tile([S, N], F32, tag="lwpre")
            nc.vector.scalar_tensor_tensor(
                lwpre, dms, dpos_m, pm, op0=ALU.subtract, op1=ALU.mult)
            stacked = sb.tile([S, 2], F32, tag="stacked")
            lw = sb.tile([S, N], F32, tag="LW")
            nc.vector.tensor_scalar(
                lw, lwpre, 0.0, 0.0, op0=ALU.max, op1=ALU.add,
                accum_out=stacked[:, 0:1])
            c01 = sb.tile([S, N], F32, tag="C01")
            nc.vector.tensor_scalar(
                c01, lwpre, EPS, 0.0, op0=ALU.is_gt, op1=ALU.add,
                accum_out=stacked[:, 1:2])

            # ---- cross-partition reduce: out[0,:] = sum_a stacked[a,:] ----
            out_ps = ps.tile([1, 2], F32, tag="out")
            nc.tensor.matmul(out_ps, ones_col, stacked, start=True, stop=True)
            outs = sb.tile([1, 2], F32, tag="outs")
            nc.scalar.copy(outs, out_ps)
            nc.sync.dma_start(out=out[:, :], in_=outs)

    nc.finalize()
    return nc


_NC_CACHE: dict = {}


def _get_nc() -> bass.Bass:
    if "nc" not in _NC_CACHE:
        _NC_CACHE["nc"] = build_nc()
    return _NC_CACHE["nc"]


def make_in_maps(output1, output2, weight):
    o1 = np.asarray(output1, dtype=np.float32)
    o2 = np.asarray(output2, dtype=np.float32)
    w = np.asarray(weight, dtype=np.float32)

    emb = np.concatenate([o1, o2], axis=0)  # (384, 512) unnormalized
    aS = np.arange(S)

    in_maps = []
    for c in range(NCORES):
        rot = (np.arange(N) + c * S) % N                  # local -> global
        embr = np.ascontiguousarray(emb[rot])             # (384, 512)
        embt = np.ascontiguousarray(embr.T)               # (512, 384)
        pmw = np.ascontiguousarray(w[rot[:S] % B][:, rot % B])  # (48, 384)
        pmw[aS, aS] = 0.0          # k == i
        pmw[aS, B + aS] = 0.0      # k == p(i)
        in_maps.append({"embt": embt, "embr": embr, "pmwn": -pmw})
    return in_maps


def reduce_outputs(results):
    parts = np.stack([r["out"][0] for r in results])      # (8, 2)
    total = parts.sum(axis=0, dtype=np.float32)
    return np.asarray(
        np.float32(total[0]) / (np.float32(total[1]) + np.float32(EPS)),
        dtype=np.float32)


def kernel(output1, output2, weight):
    in_maps = make_in_maps(output1, output2, weight)
    res = run_bass_kernel_spmd(_get_nc(), in_maps, core_ids=list(range(NCORES)))
    return reduce_outputs(res.results)



# revision 4
# speedup vs baseline: 1.1155x; 1.1155x over previous
"""BatchAll triplet loss (multi-module variant) on 8 Trainium2 NeuronCores.

Math: labels = [0..191, 0..191] -- every label appears exactly twice, so each
anchor i has exactly ONE valid positive j = (i+192) % 384.  The (i,j,k) cubic
triplet tensor collapses to an (i,k) problem:

    loss_terms[i,k] = relu(d(i, p(i)) - d(i,k) + margin) * pm[i,k] * valid[i,k]
    out = sum(loss_terms) / (count(loss_terms > EPS) + EPS)

where valid excludes k in {i, p(i)} and pm = tile(weight, (2,2)).

With G the raw Gram matrix and rn = 1/||e||:
    d(i,k) = sqrt(max(2 + t1[i,k], 0)),   t1[i,k] = -2 * rn_i * rn_k * G[i,k]

Weighting trick: with pmn = -pm,  relu(dpos+m-d)*pm == max((d-(dpos+m))*pmn, 0)
and count(lw > EPS) == count((d-(dpos+m))*pmn > EPS).

Sharding: anchors i are blocked over the 8 cores (48 each).  Each core gets the
full embedding set ROTATED so its anchor slab is local indices 0..47 and the
positives are at 192..239; shipped in bf16 in BOTH layouts (row-major for the
norms, D-major for the Gram), pre-tiled into [128, x] SBUF images so one DMA
covers several 128-row chunks.  Each core emits per-anchor [sum, count] partials
([48,2]); the host reduces all 8*48 rows and forms sum/(count+EPS).

bf16 notes: inputs are ~N(0,1); the Gram/norms are bf16 x bf16 -> f32 PSUM
accumulation, distances carry ~0.1% relative noise into a 2e-2 tolerance.

Perf structure (from NTFF traces of the f32 baseline):
- bf16 halves DMA bytes and runs the PE at full rate (fp32 matmul is 1/4 rate).
- 4 big input DMAs (2 per HWDGE ring) instead of 8: issue cost is ~0.6us each
  on the issuing sequencer.
- norms path: row-layout squares reduce on DVE as tiles land; rn transposed to
  rows in ONE PE op ([128,3] -> [3,128]) + ONE DVE copy; rank-1 broadcast of
  rn_k via 3 tiny PE matmuls.
- t1 = (rb * (-2 rn_a)) * G fused into one scalar_tensor_tensor; G is copied
  PSUM->SBUF on the (otherwise idle) ACT engine so the stt has only one PSUM
  operand.
- final sums: relu+accum on ACT in parallel with is_gt+accum on DVE.
- a dummy sqrt right at kernel start pulls the 1.3us ACT table load into the
  DMA shadow; dummy matmuls keep the PE HAM-unthrottled before the Gram.
"""

import os
import sys

for _p in ("/opt/trn_rl_repo", "/root/.axon_site/_ro/trn_rl_repo"):
    if _p not in sys.path:
        sys.path.append(_p)

# The SPMD dispatch path (bass2jax.run_bass_via_pjrt) takes jax.devices(), so
# the axon platform must stay visible.  If jax has not been initialized yet and
# JAX_PLATFORMS would hide it (e.g. "cpu"), clear the restriction.
if "jax" not in sys.modules and os.environ.get("JAX_PLATFORMS") in ("cpu",):
    del os.environ["JAX_PLATFORMS"]

import ml_dtypes
import numpy as np

import concourse.bass as bass
import concourse.tile as tile
from concourse import mybir
from concourse.bacc import Bacc
from concourse.bass_utils import run_bass_kernel_spmd

# The walrus NEFF epilogue resets every semaphore 3..255 one instruction at a
# time, split across the 5 engines (~6us, ~45% of kernel wall time).  The
# kernel itself only touches sems >= 150 (bass range) and the tile framework
# range-clears those itself at tile exit, so capping the compiler's sem space
# shrinks the reset storm.  Opt-in via env while validating.
_MAX_SEM = os.environ.get("KERNEL_MAX_SEM_NUM")
if _MAX_SEM:
    try:
        import concourse.bass_utils as _bu

        if not getattr(_bu, "_max_sem_patched", False):
            _orig_gwa = _bu.get_walrus_args

            def _gwa(*a, **k):
                return _orig_gwa(*a, **k) + [f"--max-sem-num={_MAX_SEM}"]

            _bu.get_walrus_args = _gwa
            _bu._max_sem_patched = True
    except Exception:
        pass

F32 = mybir.dt.float32
BF16 = mybir.dt.bfloat16
ALU = mybir.AluOpType
ACT = mybir.ActivationFunctionType

B = 192          # batch (distinct labels)
N = 2 * B        # embeddings
D = 512          # embedding dim
NCORES = 8
S = N // NCORES  # anchors per core (48)
MARGIN = 0.1
EPS = 1e-8
N_WARMUP = 6     # dummy matmuls to keep the PE out of HAM throttle
RT = N // 128    # row-layout chunks (3)
KC = D // 128    # contraction chunks (4)


def build_nc() -> bass.Bass:
    nc = Bacc()

    embr = nc.dram_tensor("embr", [128, RT * D], BF16, kind="ExternalInput")
    embt = nc.dram_tensor("embt", [128, KC * N], BF16, kind="ExternalInput")
    pmwn = nc.dram_tensor("pmwn", [S, N], BF16, kind="ExternalInput")
    out = nc.dram_tensor("out", [S, 2], F32, kind="ExternalOutput")

    with tile.TileContext(nc) as tc:
        with (
            tc.tile_pool(name="sb", bufs=1) as sb,
            tc.tile_pool(name="ps", bufs=1, space="PSUM") as ps,
        ):
            Rb = sb.tile([128, RT * D], BF16, tag="Rb")
            Eb = sb.tile([128, KC * N], BF16, tag="Eb")
            pm = sb.tile([S, N], BF16, tag="pm")

            # ---- loads: R chunks first (norm path is the long pole), split
            #      across both HWDGE rings; pm via SWDGE (needed late) ----
            nc.sync.dma_start(out=Rb[:, 0:2 * D], in_=embr[:, 0:2 * D])
            nc.scalar.dma_start(out=Rb[:, 2 * D:3 * D], in_=embr[:, 2 * D:3 * D])
            nc.scalar.dma_start(out=Eb[:, 0:2 * N], in_=embt[:, 0:2 * N])
            nc.sync.dma_start(out=Eb[:, 2 * N:4 * N], in_=embt[:, 2 * N:4 * N])
            nc.gpsimd.dma_start(out=pm, in_=pmwn[:, :])

            # ---- identity via iotas on gpsimd + is_equal on DVE ----
            icol = sb.tile([128, 128], F32, tag="icol")
            nc.gpsimd.iota(icol, [[1, 128]], channel_multiplier=0,
                           allow_small_or_imprecise_dtypes=True)
            iprt = sb.tile([128, 1], F32, tag="iprt")
            nc.gpsimd.iota(iprt, [[0, 1]], channel_multiplier=1,
                           allow_small_or_imprecise_dtypes=True)
            ident = sb.tile([128, 128], F32, tag="ident")
            nc.vector.tensor_scalar(ident, icol, iprt, None, op0=ALU.is_equal)

            # ---- consts / warm-up scaffolding ----
            wt = sb.tile([128, D], BF16, tag="wt")
            nc.vector.memset(wt, 1.0)
            ones_row = sb.tile([1, S], BF16, tag="ones_row")
            nc.vector.memset(ones_row, 1.0)
            twos_col = sb.tile([S, 1], F32, tag="twos_col")
            nc.vector.memset(twos_col, 2.0)
            tdum = sb.tile([1, 1], F32, tag="tdum")
            nc.scalar.sqrt(tdum, twos_col[0:1, 0:1])  # pull ACT sqrt table early

            wps = ps.tile([128, D], F32, tag="wps")
            for _ in range(N_WARMUP):
                nc.tensor.matmul(wps, wt[:, 0:128], wt, start=True, stop=True)

            # ---- norms on DVE: ns_col[p,t] = ||emb[128t+p]||^2; order 2,0,1
            #      (chunk 2 rides the smaller scalar-ring DMA, lands first) ----
            ns_col = sb.tile([128, RT], F32, tag="ns_col")
            junk = sb.tile([128, D], BF16, tag="junk")
            for t in (2, 0, 1):
                nc.vector.scalar_tensor_tensor(
                    junk, Rb[:, t * D:(t + 1) * D], 1.0, Rb[:, t * D:(t + 1) * D],
                    op0=ALU.mult, op1=ALU.mult, accum_out=ns_col[:, t:t + 1])
            nrm = sb.tile([128, RT], F32, tag="nrm")
            nc.scalar.sqrt(nrm, ns_col)
            rn_col = sb.tile([128, RT], F32, tag="rn_col")   # 1/||e||
            nc.vector.reciprocal(rn_col, nrm)
            rnam2 = sb.tile([S, 1], F32, tag="rnam2")        # -2 * rn[slab]
            nc.vector.tensor_scalar_mul(rnam2, rn_col[0:S, 0:1], -2.0)

            # ---- Gram slab: G[a,k] = sum_d embt[d,a] * embt[d,k];
            #      chunks 2,3 first (their DMA lands first) ----
            g_ps = ps.tile([S, N], F32, tag="G")
            for i, c in enumerate((2, 3, 0, 1)):
                nc.tensor.matmul(g_ps, Eb[:, c * N:c * N + S],
                                 Eb[:, c * N:(c + 1) * N],
                                 start=(i == 0), stop=(i == KC - 1))

            # ---- rn to a single [1, N] row: 3 transposes (each lands at
            #      partition 0) into one PSUM row + ONE copy on ACT ----
            rnT_ps = ps.tile([1, N], F32, tag="rnT")
            for j in range(RT):
                nc.tensor.transpose(rnT_ps[:, j * 128:(j + 1) * 128],
                                    rn_col[:, j:j + 1], ident)
            rn_row = sb.tile([1, N], BF16, tag="rn_row")
            nc.scalar.copy(rn_row, rnT_ps)

            # ---- RB[a,k] = rn_k broadcast (rank-1, 128 cols per chunk) ----
            rb_ps = ps.tile([S, N], F32, tag="RB")
            for j in range(RT):
                nc.tensor.matmul(rb_ps[:, j * 128:(j + 1) * 128], ones_row,
                                 rn_row[:, j * 128:(j + 1) * 128],
                                 start=True, stop=True)

            # ---- G to SBUF on ACT (so t1 has a single PSUM operand) ----
            gs = sb.tile([S, N], BF16, tag="gs")
            nc.scalar.copy(gs, g_ps)

            # ---- t1 = (RB * -2rn_a) * G ----
            t1 = sb.tile([S, N], BF16, tag="T1")
            nc.vector.scalar_tensor_tensor(
                t1, rb_ps, rnam2, gs, op0=ALU.mult, op1=ALU.mult)

            # ---- positive distance from t1's diagonal block ----
            dpb = sb.tile([S, S], BF16, tag="dpb")
            t1pos = sb.tile([S, 1], F32, tag="t1pos")
            nc.vector.scalar_tensor_tensor(
                dpb, t1[:, B:B + S], 1.0, ident[0:S, 0:S], op0=ALU.mult,
                op1=ALU.mult, accum_out=t1pos)
            d2 = sb.tile([S, N], BF16, tag="D2")
            nc.vector.tensor_scalar(d2, t1, 2.0, 0.0, op0=ALU.add, op1=ALU.max)
            dpos = sb.tile([S, 1], F32, tag="dpos")
            nc.scalar.activation(dpos, t1pos, ACT.Sqrt, bias=twos_col, scale=1.0)
            dpos_m = sb.tile([S, 1], F32, tag="dpos_m")
            nc.vector.tensor_scalar_add(dpos_m, dpos, MARGIN)
            dms = sb.tile([S, N], BF16, tag="dms")
            nc.scalar.sqrt(dms, d2)

            # ---- weighted triplet terms via the negated-weight trick ----
            lwpre = sb.tile([S, N], BF16, tag="lwpre")
            nc.vector.scalar_tensor_tensor(
                lwpre, dms, dpos_m, pm, op0=ALU.subtract, op1=ALU.mult)

            # ---- per-anchor [sum, count]: relu+accum on ACT, is_gt on DVE ----
            stacked = sb.tile([S, 2], F32, tag="stacked")
            lw = sb.tile([S, N], BF16, tag="LW")
            nc.scalar.activation(lw, lwpre, ACT.Relu,
                                 accum_out=stacked[:, 0:1])
            c01 = sb.tile([S, N], BF16, tag="C01")
            nc.vector.tensor_scalar(
                c01, lwpre, EPS, 0.0, op0=ALU.is_gt, op1=ALU.add,
                accum_out=stacked[:, 1:2])

            nc.sync.dma_start(out=out[:, :], in_=stacked)

    nc.finalize()
    return nc


_NC_CACHE: dict = {}


def _get_nc() -> bass.Bass:
    if "nc" not in _NC_CACHE:
        _NC_CACHE["nc"] = build_nc()
    return _NC_CACHE["nc"]


def make_in_maps(output1, output2, weight):
    o1 = np.asarray(output1, dtype=np.float32)
    o2 = np.asarray(output2, dtype=np.float32)
    w = np.asarray(weight, dtype=np.float32)

    emb = np.concatenate([o1, o2], axis=0)  # (384, 512) unnormalized
    aS = np.arange(S)

    in_maps = []
    for c in range(NCORES):
        rot = (np.arange(N) + c * S) % N                  # local -> global
        er = emb[rot].astype(ml_dtypes.bfloat16)          # (384, 512)
        # row-layout image: [128, 3*512], chunk t at cols [512t, 512t+512)
        embr_h = np.ascontiguousarray(
            er.reshape(RT, 128, D).transpose(1, 0, 2).reshape(128, RT * D))
        # D-major image: [128, 4*384], chunk c at cols [384c, 384c+384)
        et = np.ascontiguousarray(er.T)                   # (512, 384)
        embt_h = np.ascontiguousarray(
            et.reshape(KC, 128, N).transpose(1, 0, 2).reshape(128, KC * N))
        pmw = w[rot[:S] % B][:, rot % B].astype(np.float32)  # (48, 384)
        pmw[aS, aS] = 0.0          # k == i
        pmw[aS, B + aS] = 0.0      # k == p(i)
        in_maps.append({
            "embr": embr_h,
            "embt": embt_h,
            "pmwn": (-pmw).astype(ml_dtypes.bfloat16),
        })
    return in_maps


def reduce_outputs(results):
    parts = np.stack([r["out"] for r in results])         # (8, 48, 2)
    total = parts.sum(axis=(0, 1), dtype=np.float32)
    return np.asarray(
        np.float32(total[0]) / (np.float32(total[1]) + np.float32(EPS)),
        dtype=np.float32)


def kernel(output1, output2, weight):
    in_maps = make_in_maps(output1, output2, weight)
    res = run_bass_kernel_spmd(_get_nc(), in_maps, core_ids=list(range(NCORES)))
    return reduce_outputs(res.results)
